# revision 63
# baseline (speedup 1.0000x reference)
"""Trainium2 Bass kernel for the ChessformerAdapter problem.

Computation (per batch row b of obs (B,15,8,8) f32):
  - tokens: for each of 64 squares, first plane p (of 12) with obs>0.5 wins
    -> token = p+1, else 0
  - scores = emb[token] -> from_scores (64,), to_scores (64,)
  - logits[f*64+t] = from[f] + to[t], diagonal (f==t) zeroed  -> (4096,)
  - cols 4096..4099 = max(from_scores)
  - values = zeros (B,)

Sharding: pure data parallel over 8 NeuronCores, 2048 batch rows each.
The 13x2 emb table is replicated (host pre-broadcast to 128 partitions).

Device algorithm per mega-tile (G groups of 128 batch rows; batch on
partitions, groups along the free dim):
  - one u8 mask tensor for all 12 planes: (obs > 0.5)
  - score tiles start at emb[0]; for p = 11..0 copy_predicated overwrites
    with emb[p+1] where mask_p -> first plane wins (later calls win)
  - outer add from[f]+to[t] via broadcast-AP tensor_tensor, split across
    DVE (f < FSPLIT) and GPSIMD (f >= FSPLIT), in half-tiles of GH groups
    so output DMAs pipeline finely
  - diagonal zero via strided memset (GPSIMD), promo via reduce_max +
    broadcast copy (DVE)
"""

import numpy as np

import bass_rust
import concourse.bass as bass
import concourse.mybir as mybir
from concourse.bass_utils import run_bass_kernel_spmd
from concourse.tile import TileContext

N_CORES = 8
B_FULL = 16384
B_CORE = B_FULL // N_CORES  # 2048
P = 128  # partitions
NG = B_CORE // P  # 16 groups of 128 batch rows per core
# (start, count) group schedule: small first/last mega-tiles shrink
# pipeline fill/drain; large middle ones keep the small-op count down.
MEGAS = [(0, 2), (2, 2), (4, 4), (8, 4), (12, 4)]
GH = 2  # groups per output half-tile
NPL = 12  # piece planes
NSQ = 64
NLOG = 4100
OBS_ROW = 15 * 64  # 960
FSPLIT = 32  # outer-add rows on DVE; the rest on GPSIMD
F32 = mybir.dt.float32
U8 = mybir.dt.uint8


def build_nc() -> bass.Bass:
    nc = bass.Bass()
    obs_d = nc.declare_dram_parameter("obs", [B_CORE, OBS_ROW], F32, isOutput=False)
    emb_d = nc.declare_dram_parameter("embrep", [P, 26], F32, isOutput=False)
    log_d = nc.declare_dram_parameter("logits", [B_CORE, NLOG], F32, isOutput=True)
    val_d = nc.declare_dram_parameter("values", [B_CORE], F32, isOutput=True)

    obs_v = obs_d[:].rearrange("(g p) c -> p g c", p=P)
    log_v = log_d[:].rearrange("(g p) c -> p g c", p=P)

    with TileContext(nc) as tc:
        with (
            tc.tile_pool(name="const", bufs=1) as cpool,
            tc.tile_pool(name="obs", bufs=len(MEGAS)) as opool,
            tc.tile_pool(name="mask", bufs=2) as mpool,
            tc.tile_pool(name="score", bufs=3) as spool,
            tc.tile_pool(name="promo", bufs=2) as ppool,
            tc.tile_pool(name="pscore", bufs=3) as pspool,
            tc.tile_pool(name="logit", bufs=3) as lpool,
        ):
            all_dmas = []
            emb_sb = cpool.tile([P, 26], F32)
            all_dmas.append(nc.sync.dma_start(out=emb_sb[:], in_=emb_d[:]))

            zv = cpool.tile([1, B_CORE], F32)
            nc.vector.memset(zv[:], 0.0)
            all_dmas.append(
                nc.sync.dma_start(
                    out=val_d[:].rearrange("(a b) -> a b", a=1), in_=zv[:]
                )
            )

            def emit_chain(g0, gcnt, absorb=()):
                """Score stage: obs DMA, plane masks, first-wins CP chain.

                ``absorb``: instructions whose completion sems the DVE must
                observe before this mega's output stage (reused logit-slot
                WAR/WAW). Each is attached to its own CP instruction — a
                data ancestor of the tensor_tensor consumers with no other
                waits — so no instruction exceeds the 1-sync-wait HW limit.
                """
                gn = gcnt * NSQ
                obs_t = opool.tile([P, gcnt * NPL * NSQ], F32)
                all_dmas.append(
                    nc.sync.dma_start(
                        out=obs_t[:].rearrange("p (g c) -> p g c", g=gcnt),
                        in_=obs_v[:, g0 : g0 + gcnt, 0 : NPL * NSQ],
                    )
                )
                obs4 = obs_t[:].rearrange("p (g q s) -> p g q s", g=gcnt, q=NPL)

                # all 12 plane masks in one op
                mk = mpool.tile([P, gcnt * NPL * NSQ], U8)
                nc.vector.tensor_scalar(
                    out=mk[:].rearrange("p (g q s) -> p g q s", g=gcnt, q=NPL),
                    in0=obs4,
                    scalar1=0.5,
                    scalar2=None,
                    op0=mybir.AluOpType.is_gt,
                )
                # (p, s, g) views: transposed so none of the CP operand APs
                # collapse — the simulator requires identical view shapes.
                mk_t = mk[:].rearrange("p (g q s) -> p q s g", g=gcnt, q=NPL)

                from_t = spool.tile([P, gn], F32)
                to_t = spool.tile([P, gn], F32)
                nc.vector.tensor_copy(
                    out=from_t[:], in_=emb_sb[:, 0:1].broadcast_to((P, gn))
                )
                nc.vector.tensor_copy(
                    out=to_t[:], in_=emb_sb[:, 1:2].broadcast_to((P, gn))
                )
                from_sg = from_t[:].rearrange("p (g s) -> p s g", g=gcnt)
                to_sg = to_t[:].rearrange("p (g s) -> p s g", g=gcnt)
                cps = []
                for p in reversed(range(NPL)):
                    mk_p = mk_t[:, p]
                    cps.append(
                        nc.vector.copy_predicated(
                            out=from_sg,
                            mask=mk_p,
                            data=emb_sb[:, 2 * (p + 1) : 2 * (p + 1) + 1]
                            .unsqueeze(2)
                            .broadcast_to((P, NSQ, gcnt)),
                        )
                    )
                    cps.append(
                        nc.vector.copy_predicated(
                            out=to_sg,
                            mask=mk_p,
                            data=emb_sb[:, 2 * (p + 1) + 1 : 2 * (p + 1) + 2]
                            .unsqueeze(2)
                            .broadcast_to((P, NSQ, gcnt)),
                        )
                    )
                return from_t, to_t

            out_dmas = []
            memsets = []
            last_dve = None
            lbufs = 3  # lpool bufs
            li = 0
            for mi, (g0, gcnt) in enumerate(MEGAS):
                nh = gcnt // GH
                from_t, to_t = emit_chain(g0, gcnt)
                fr3 = from_t[:].rearrange("p (g f) -> p g f", g=gcnt)
                to3 = to_t[:].rearrange("p (g t) -> p g t", g=gcnt)

                for h in range(nh):
                    gsl = slice(h * GH, (h + 1) * GH)
                    npf = NSQ - FSPLIT

                    lt = lpool.tile([P, GH * NLOG], F32)
                    l3 = lt[:].rearrange("p (g c) -> p g c", g=GH)
                    # Guard memset: the slot's FIRST writer, so the reused
                    # slot's WAR wait (old out-DMA completion sem) lands
                    # here alone; all later writers are covered by DVE
                    # program order. Keeps every instruction at <=1 sync
                    # wait (the TRN2 codegen limit).
                    nc.vector.memset(lt[0:1, 0:1], 0.0)
                    # outer add split across DVE (f < FSPLIT) and GPSIMD
                    # (f >= FSPLIT); with FSPLIT=NSQ everything stays on the
                    # DVE, which keeps every instruction at <=1 sync wait
                    # (a logit tile written by two engines forces 2-wait
                    # instructions somewhere, which TRN2 codegen rejects).
                    nc.vector.tensor_tensor(
                        out=l3[:, :, 0 : FSPLIT * NSQ].rearrange(
                            "p g (f t) -> p g f t", f=FSPLIT
                        ),
                        in0=fr3[:, gsl, 0:FSPLIT]
                        .unsqueeze(3)
                        .broadcast_to((P, GH, FSPLIT, NSQ)),
                        in1=to3[:, gsl]
                        .unsqueeze(2)
                        .broadcast_to((P, GH, FSPLIT, NSQ)),
                        op=mybir.AluOpType.add,
                    )
                    if FSPLIT < NSQ:
                        pfrom = pspool.tile([P, GH * npf], F32)
                        pto = pspool.tile([P, GH * NSQ], F32)
                        nc.gpsimd.tensor_copy(
                            out=pfrom[:].rearrange("p (g f) -> p g f", g=GH),
                            in_=fr3[:, gsl, FSPLIT:],
                        )
                        pc2 = nc.gpsimd.tensor_copy(
                            out=pto[:].rearrange("p (g t) -> p g t", g=GH),
                            in_=to3[:, gsl],
                        )
                        if li >= lbufs:
                            bass_rust.add_dep_helper(
                                pc2.ins,
                                out_dmas[li - lbufs].ins,
                                sync=True,
                                reason="POOL absorbs reused logit-slot sem",
                            )
                        nc.gpsimd.tensor_tensor(
                            out=l3[:, :, FSPLIT * NSQ : 4096].rearrange(
                                "p g (f t) -> p g f t", f=npf
                            ),
                            in0=pfrom[:]
                            .rearrange("p (g f) -> p g f", g=GH)
                            .unsqueeze(3)
                            .broadcast_to((P, GH, npf, NSQ)),
                            in1=pto[:]
                            .rearrange("p (g t) -> p g t", g=GH)
                            .unsqueeze(2)
                            .broadcast_to((P, GH, npf, NSQ)),
                            op=mybir.AluOpType.add,
                        )
                    memsets.append(nc.vector.memset(l3[:, :, 0:4096:65], 0.0))

                    pr = ppool.tile([P, GH], F32)
                    nc.vector.tensor_reduce(
                        out=pr[:],
                        in_=fr3[:, gsl],
                        axis=mybir.AxisListType.X,
                        op=mybir.AluOpType.max,
                    )
                    last_dve = nc.vector.tensor_copy(
                        out=l3[:, :, 4096:4100],
                        in_=pr[:].unsqueeze(2).broadcast_to((P, GH, 4)),
                    )

                    out_dmas.append(
                        nc.gpsimd.dma_start(
                            out=log_v[:, g0 + h * GH : g0 + (h + 1) * GH, :],
                            in_=lt[:].rearrange("p (g c) -> p g c", g=GH),
                        )
                    )
                    all_dmas.append(out_dmas[-1])
                    li += 1

            # Tail: absorb each DMA-completion sem into its own SP nop.
            # Codegen allows only a couple of sync waits per instruction;
            # without this the kernel-tail drain carries one wait per
            # outstanding DMA sem lane and fails to compile.
            for d in all_dmas:
                n = nc.sync.nop(nofuse=True)
                bass_rust.add_dep_helper(
                    n.ins, d.ins, sync=True, reason="split tail drain waits"
                )
            # The drain also needs the DVE and Pool engine ticks — give each
            # its own SP nop as well.
            for last in (memsets[-1], last_dve):
                n = nc.sync.nop(nofuse=True)
                bass_rust.add_dep_helper(
                    n.ins, last.ins, sync=True, reason="split tail drain waits"
                )
    return nc


_NC_CACHE = None


def _get_nc() -> bass.Bass:
    global _NC_CACHE
    if _NC_CACHE is None:
        _NC_CACHE = build_nc()
    return _NC_CACHE


def run(obs: np.ndarray, emb: np.ndarray, **spmd_kwargs):
    """Run the kernel; returns (logits, values, BassKernelResults)."""
    obs = np.ascontiguousarray(obs, dtype=np.float32).reshape(B_FULL, OBS_ROW)
    embrep = np.ascontiguousarray(
        np.broadcast_to(np.asarray(emb, dtype=np.float32).reshape(1, 26), (P, 26))
    )
    nc = _get_nc()
    in_maps = [
        {"obs": obs[i * B_CORE : (i + 1) * B_CORE], "embrep": embrep}
        for i in range(N_CORES)
    ]
    res = run_bass_kernel_spmd(nc, in_maps, list(range(N_CORES)), **spmd_kwargs)
    logits = np.concatenate([res.results[i]["logits"] for i in range(N_CORES)], axis=0)
    values = np.concatenate([res.results[i]["values"] for i in range(N_CORES)], axis=0)
    return logits, values, res


def kernel(obs: np.ndarray, emb: np.ndarray):
    logits, values, _ = run(obs, emb)
    return logits, values


# revision 67
# speedup vs baseline: 1.0981x; 1.0981x over previous
"""Trainium2 Bass kernel for the ChessformerAdapter problem.

Computation (per batch row b of obs (B,15,8,8) f32):
  - tokens: for each of 64 squares, first plane p (of 12) with obs>0.5 wins
    -> token = p+1, else 0
  - scores = emb[token] -> from_scores (64,), to_scores (64,)
  - logits[f*64+t] = from[f] + to[t], diagonal (f==t) zeroed  -> (4096,)
  - cols 4096..4099 = max(from_scores)
  - values = zeros (B,)

Sharding: pure data parallel over 8 NeuronCores, 2048 batch rows each.
The 13x2 emb table is replicated (host pre-broadcast to 128 partitions).

Per mega-tile (gcnt groups of 128 batch rows; batch on partitions):
  - one u8 mask tensor for all 12 planes: (obs > 0.5)
  - score tiles start at emb[0]; for p = 11..0 copy_predicated overwrites
    with emb[p+1] where mask_p -> first plane wins (later calls win)
  - outer add from[f]+to[t] via broadcast-AP tensor_tensor, split across
    two single-writer-engine tiles: DVE owns HBM cols [0, FSPLIT*64),
    GPSIMD owns the rest (incl. its diag slice and the promo cols)

Every instruction carries <=1 sync wait (TRN2 codegen limit): one writer
engine per DMA'd tile, guard memsets absorb reused-slot WAR sems, and DMA
counts fit the sem-lane budgets exactly (8 HWDGE: 3 obs + 5 ltd; 7 SWDGE:
5 ltp + emb + values).
"""

import numpy as np

import bass_rust
import concourse.bass as bass
import concourse.mybir as mybir
from concourse.bass_utils import run_bass_kernel_spmd
from concourse.tile import TileContext

N_CORES = 8
B_FULL = 16384
B_CORE = B_FULL // N_CORES  # 2048
P = 128  # partitions
NG = B_CORE // P  # 16 groups of 128 batch rows per core
# (start, count) group schedule: small first mega-tiles shrink pipeline
# fill; large later ones keep the small-op count down.
MEGAS = [(0, 2), (2, 2), (4, 4), (8, 4), (12, 4)]
# obs input pieces (start, count) and per-mega piece index
OBS_PIECES = [(0, 2), (2, 2), (4, 12)]
MEGA_PIECE = [0, 1, 2, 2, 2]
NPL = 12  # piece planes
NSQ = 64
NLOG = 4100
OBS_ROW = 15 * 64  # 960
FSPLIT = 36  # outer-add rows on DVE; the rest on GPSIMD
DCOL = FSPLIT * NSQ  # 2304
PCOL = NLOG - DCOL  # 1796
NPF = NSQ - FSPLIT  # 28
F32 = mybir.dt.float32
U8 = mybir.dt.uint8


def build_nc() -> bass.Bass:
    nc = bass.Bass()
    obs_d = nc.declare_dram_parameter("obs", [B_CORE, OBS_ROW], F32, isOutput=False)
    emb_d = nc.declare_dram_parameter("embrep", [P, 26], F32, isOutput=False)
    log_d = nc.declare_dram_parameter("logits", [B_CORE, NLOG], F32, isOutput=True)
    val_d = nc.declare_dram_parameter("values", [B_CORE], F32, isOutput=True)

    obs_v = obs_d[:].rearrange("(g p) c -> p g c", p=P)
    log_v = log_d[:].rearrange("(g p) c -> p g c", p=P)

    with TileContext(nc) as tc:
        with (
            tc.tile_pool(name="const", bufs=1) as cpool,
            tc.tile_pool(name="obs", bufs=1) as opool,
            tc.tile_pool(name="mask", bufs=2) as mpool,
            tc.tile_pool(name="score", bufs=2) as spool,
            tc.tile_pool(name="promo", bufs=5) as ppool,
            tc.tile_pool(name="ltp", bufs=2) as pspool,
            tc.tile_pool(name="ltd", bufs=2) as lpool,
        ):
            all_dmas = []
            emb_sb = cpool.tile([P, 26], F32)
            all_dmas.append(nc.gpsimd.dma_start(out=emb_sb[:], in_=emb_d[:]))

            # values: zeros, laid out (16 partitions x 128 contiguous)
            zv = cpool.tile([16, P], F32)
            nc.vector.memset(zv[:], 0.0)
            all_dmas.append(
                nc.gpsimd.dma_start(
                    out=val_d[:].rearrange("(g p) -> g p", p=P), in_=zv[:]
                )
            )

            # obs input in 3 HWDGE DMAs: two small pieces feed the small
            # lead-in mega-tiles quickly, one large piece covers the rest
            obs_tiles = []
            for pi, (og0, ocnt) in enumerate(OBS_PIECES):
                ot = opool.tile([P, ocnt * NPL * NSQ], F32, tag=f"obs{pi}")
                all_dmas.append(
                    nc.sync.dma_start(
                        out=ot[:].rearrange("p (g c) -> p g c", g=ocnt),
                        in_=obs_v[:, og0 : og0 + ocnt, 0 : NPL * NSQ],
                    )
                )
                obs_tiles.append(
                    ot[:].rearrange("p (g q s) -> p g q s", g=ocnt, q=NPL)
                )

            def emit_chain(gcnt, obs4):
                """Score stage: plane masks + first-wins CP chain."""
                gn = gcnt * NSQ
                # all 12 plane masks in one op
                mk = mpool.tile([P, gcnt * NPL * NSQ], U8)
                nc.vector.tensor_scalar(
                    out=mk[:].rearrange("p (g q s) -> p g q s", g=gcnt, q=NPL),
                    in0=obs4,
                    scalar1=0.5,
                    scalar2=None,
                    op0=mybir.AluOpType.is_gt,
                )
                # (p, s, g) views: transposed so none of the CP operand APs
                # collapse — the simulator requires identical view shapes.
                mk_t = mk[:].rearrange("p (g q s) -> p q s g", g=gcnt, q=NPL)

                from_t = spool.tile([P, gn], F32)
                to_t = spool.tile([P, gn], F32)
                nc.vector.tensor_copy(
                    out=from_t[:], in_=emb_sb[:, 0:1].broadcast_to((P, gn))
                )
                nc.vector.tensor_copy(
                    out=to_t[:], in_=emb_sb[:, 1:2].broadcast_to((P, gn))
                )
                from_sg = from_t[:].rearrange("p (g s) -> p s g", g=gcnt)
                to_sg = to_t[:].rearrange("p (g s) -> p s g", g=gcnt)
                for p in reversed(range(NPL)):
                    mk_p = mk_t[:, p]
                    nc.vector.copy_predicated(
                        out=from_sg,
                        mask=mk_p,
                        data=emb_sb[:, 2 * (p + 1) : 2 * (p + 1) + 1]
                        .unsqueeze(2)
                        .broadcast_to((P, NSQ, gcnt)),
                    )
                    nc.vector.copy_predicated(
                        out=to_sg,
                        mask=mk_p,
                        data=emb_sb[:, 2 * (p + 1) + 1 : 2 * (p + 1) + 2]
                        .unsqueeze(2)
                        .broadcast_to((P, NSQ, gcnt)),
                    )
                return from_t, to_t

            last_pool = None
            memsets = []
            for mi, (g0, gcnt) in enumerate(MEGAS):
                piece = MEGA_PIECE[mi]
                pg0 = g0 - OBS_PIECES[piece][0]
                obs4 = obs_tiles[piece][:, pg0 : pg0 + gcnt]
                from_t, to_t = emit_chain(gcnt, obs4)
                fr3 = from_t[:].rearrange("p (g f) -> p g f", g=gcnt)
                to3 = to_t[:].rearrange("p (g t) -> p g t", g=gcnt)

                ltd = lpool.tile([P, gcnt * DCOL], F32)
                ltp = pspool.tile([P, gcnt * PCOL], F32)
                ld3 = ltd[:].rearrange("p (g c) -> p g c", g=gcnt)
                lp3 = ltp[:].rearrange("p (g c) -> p g c", g=gcnt)
                # guard memsets: first writer of a REUSED slot takes the
                # WAR wait (old out-DMA completion sem) alone; only needed
                # once slots recycle (mi >= bufs)
                if mi >= 2:
                    nc.vector.memset(ltd[0:1, 0:1], 0.0)
                    nc.gpsimd.memset(
                        ltp[0:1, NPF * NSQ : NPF * NSQ + 1], 0.0
                    )

                nc.vector.tensor_tensor(
                    out=ld3.rearrange("p g (f t) -> p g f t", f=FSPLIT),
                    in0=fr3[:, :, 0:FSPLIT]
                    .unsqueeze(3)
                    .broadcast_to((P, gcnt, FSPLIT, NSQ)),
                    in1=to3.unsqueeze(2).broadcast_to((P, gcnt, FSPLIT, NSQ)),
                    op=mybir.AluOpType.add,
                )
                nc.gpsimd.tensor_tensor(
                    out=lp3[:, :, 0 : NPF * NSQ].rearrange(
                        "p g (f t) -> p g f t", f=NPF
                    ),
                    in0=fr3[:, :, FSPLIT:]
                    .unsqueeze(3)
                    .broadcast_to((P, gcnt, NPF, NSQ)),
                    in1=to3.unsqueeze(2).broadcast_to((P, gcnt, NPF, NSQ)),
                    op=mybir.AluOpType.add,
                )
                # diagonal zeros: f < FSPLIT live in ltd, the rest in ltp
                memsets.append(nc.vector.memset(ld3[:, :, 0:DCOL:65], 0.0))
                diag0 = FSPLIT * 65 - DCOL
                nc.gpsimd.memset(lp3[:, :, diag0 : NPF * NSQ : 65], 0.0)

                pr = ppool.tile([P, gcnt], F32)
                nc.vector.tensor_reduce(
                    out=pr[:],
                    in_=fr3,
                    axis=mybir.AxisListType.X,
                    op=mybir.AluOpType.max,
                )
                last_pool = nc.gpsimd.tensor_copy(
                    out=lp3[:, :, NPF * NSQ : PCOL],
                    in_=pr[:].unsqueeze(2).broadcast_to((P, gcnt, 4)),
                )

                # DVE-region out via scalar-engine HWDGE, POOL-region out
                # via gpsimd SWDGE — each ring stays within its 8 sem lanes
                all_dmas.append(
                    nc.scalar.dma_start(
                        out=log_v[:, g0 : g0 + gcnt, 0:DCOL], in_=ld3
                    )
                )
                all_dmas.append(
                    nc.gpsimd.dma_start(
                        out=log_v[:, g0 : g0 + gcnt, DCOL:NLOG], in_=lp3
                    )
                )

            # Tail: absorb each DMA-completion sem into its own SP nop
            # (the kernel-tail drain may carry only one wait itself)
            for d in all_dmas:
                n = nc.sync.nop(nofuse=True)
                bass_rust.add_dep_helper(
                    n.ins, d.ins, sync=True, reason="split tail drain waits"
                )
            for last in (memsets[-1], last_pool):
                n = nc.sync.nop(nofuse=True)
                bass_rust.add_dep_helper(
                    n.ins, last.ins, sync=True, reason="split tail drain waits"
                )
    return nc


_NC_CACHE = None


def _get_nc() -> bass.Bass:
    global _NC_CACHE
    if _NC_CACHE is None:
        _NC_CACHE = build_nc()
    return _NC_CACHE


def run(obs: np.ndarray, emb: np.ndarray, **spmd_kwargs):
    """Run the kernel; returns (logits, values, BassKernelResults)."""
    obs = np.ascontiguousarray(obs, dtype=np.float32).reshape(B_FULL, OBS_ROW)
    embrep = np.ascontiguousarray(
        np.broadcast_to(np.asarray(emb, dtype=np.float32).reshape(1, 26), (P, 26))
    )
    nc = _get_nc()
    in_maps = [
        {"obs": obs[i * B_CORE : (i + 1) * B_CORE], "embrep": embrep}
        for i in range(N_CORES)
    ]
    res = run_bass_kernel_spmd(nc, in_maps, list(range(N_CORES)), **spmd_kwargs)
    logits = np.concatenate([res.results[i]["logits"] for i in range(N_CORES)], axis=0)
    values = np.concatenate([res.results[i]["values"] for i in range(N_CORES)], axis=0)
    return logits, values, res


def kernel(obs: np.ndarray, emb: np.ndarray):
    logits, values, _ = run(obs, emb)
    return logits, values


# revision 70
# speedup vs baseline: 1.1593x; 1.0557x over previous
"""Trainium2 Bass kernel for the ChessformerAdapter problem.

Computation (per batch row b of obs (B,15,8,8) f32):
  - tokens: for each of 64 squares, first plane p (of 12) with obs>0.5 wins
    -> token = p+1, else 0
  - scores = emb[token] -> from_scores (64,), to_scores (64,)
  - logits[f*64+t] = from[f] + to[t], diagonal (f==t) zeroed  -> (4096,)
  - cols 4096..4099 = max(from_scores)
  - values = zeros (B,)

Sharding: pure data parallel over 8 NeuronCores, 2048 batch rows each.
The 13x2 emb table is replicated (host pre-broadcast to 128 partitions).

Per mega-tile (gcnt groups of 128 batch rows; batch on partitions):
  - one u8 mask tensor for all 12 planes: (obs > 0.5)
  - score tiles start at emb[0]; for p = 11..0 copy_predicated overwrites
    with emb[p+1] where mask_p -> first plane wins (later calls win)
  - outer add from[f]+to[t] via broadcast-AP tensor_tensor, split across
    two single-writer-engine tiles: DVE owns HBM cols [0, FSPLIT*64),
    GPSIMD owns the rest (incl. its diag slice and the promo cols)

Every instruction carries <=1 sync wait (TRN2 codegen limit): one writer
engine per DMA'd tile, guard memsets absorb reused-slot WAR sems, and DMA
counts fit the sem-lane budgets exactly (8 HWDGE: 3 obs + 5 ltd; 7 SWDGE:
5 ltp + emb + values).
"""

import numpy as np

import bass_rust
import concourse.bass as bass
import concourse.mybir as mybir
from concourse.bass_utils import run_bass_kernel_spmd
from concourse.tile import TileContext

N_CORES = 8
B_FULL = 16384
B_CORE = B_FULL // N_CORES  # 2048
P = 128  # partitions
NG = B_CORE // P  # 16 groups of 128 batch rows per core
# (start, count) group schedule: small first mega-tiles shrink pipeline
# fill; large later ones keep the small-op count down.
MEGAS = [(0, 2), (2, 2), (4, 4), (8, 4), (12, 4)]
# obs input pieces (start, count) and per-mega piece index
OBS_PIECES = [(0, 2), (2, 2), (4, 12)]
MEGA_PIECE = [0, 1, 2, 2, 2]
NPL = 12  # piece planes
NSQ = 64
NLOG = 4100
OBS_ROW = 15 * 64  # 960
FSPLIT = 28  # outer-add rows on DVE; the rest on GPSIMD
DCOL = FSPLIT * NSQ  # 2304
PCOL = NLOG - DCOL  # 1796
NPF = NSQ - FSPLIT  # 28
F32 = mybir.dt.float32
U8 = mybir.dt.uint8


def build_nc() -> bass.Bass:
    nc = bass.Bass()
    obs_d = nc.declare_dram_parameter("obs", [B_CORE, OBS_ROW], F32, isOutput=False)
    emb_d = nc.declare_dram_parameter("embrep", [P, 26], F32, isOutput=False)
    log_d = nc.declare_dram_parameter("logits", [B_CORE, NLOG], F32, isOutput=True)
    val_d = nc.declare_dram_parameter("values", [B_CORE], F32, isOutput=True)

    obs_v = obs_d[:].rearrange("(g p) c -> p g c", p=P)
    log_v = log_d[:].rearrange("(g p) c -> p g c", p=P)

    with TileContext(nc) as tc:
        with (
            tc.tile_pool(name="const", bufs=1) as cpool,
            tc.tile_pool(name="obs", bufs=1) as opool,
            tc.tile_pool(name="mask", bufs=2) as mpool,
            tc.tile_pool(name="score", bufs=2) as spool,
            tc.tile_pool(name="promo", bufs=5) as ppool,
            tc.tile_pool(name="ltp", bufs=2) as pspool,
            tc.tile_pool(name="ltd", bufs=2) as lpool,
        ):
            all_dmas = []
            emb_sb = cpool.tile([P, 26], F32)
            all_dmas.append(nc.gpsimd.dma_start(out=emb_sb[:], in_=emb_d[:]))

            # values: zeros, laid out (16 partitions x 128 contiguous)
            zv = cpool.tile([16, P], F32)
            nc.vector.memset(zv[:], 0.0)
            all_dmas.append(
                nc.gpsimd.dma_start(
                    out=val_d[:].rearrange("(g p) -> g p", p=P), in_=zv[:]
                )
            )

            # obs input in 3 HWDGE DMAs: two small pieces feed the small
            # lead-in mega-tiles quickly, one large piece covers the rest
            obs_tiles = []
            for pi, (og0, ocnt) in enumerate(OBS_PIECES):
                ot = opool.tile([P, ocnt * NPL * NSQ], F32, tag=f"obs{pi}")
                all_dmas.append(
                    nc.sync.dma_start(
                        out=ot[:].rearrange("p (g c) -> p g c", g=ocnt),
                        in_=obs_v[:, og0 : og0 + ocnt, 0 : NPL * NSQ],
                    )
                )
                obs_tiles.append(
                    ot[:].rearrange("p (g q s) -> p g q s", g=ocnt, q=NPL)
                )

            def emit_chain(gcnt, obs4):
                """Score stage: plane masks + first-wins CP chain."""
                gn = gcnt * NSQ
                # all 12 plane masks in one op
                mk = mpool.tile([P, gcnt * NPL * NSQ], U8)
                nc.vector.tensor_scalar(
                    out=mk[:].rearrange("p (g q s) -> p g q s", g=gcnt, q=NPL),
                    in0=obs4,
                    scalar1=0.5,
                    scalar2=None,
                    op0=mybir.AluOpType.is_gt,
                )
                # (p, s, g) views: transposed so none of the CP operand APs
                # collapse — the simulator requires identical view shapes.
                mk_t = mk[:].rearrange("p (g q s) -> p q s g", g=gcnt, q=NPL)

                from_t = spool.tile([P, gn], F32)
                to_t = spool.tile([P, gn], F32)
                nc.vector.tensor_copy(
                    out=from_t[:], in_=emb_sb[:, 0:1].broadcast_to((P, gn))
                )
                nc.vector.tensor_copy(
                    out=to_t[:], in_=emb_sb[:, 1:2].broadcast_to((P, gn))
                )
                from_sg = from_t[:].rearrange("p (g s) -> p s g", g=gcnt)
                to_sg = to_t[:].rearrange("p (g s) -> p s g", g=gcnt)
                for p in reversed(range(NPL)):
                    mk_p = mk_t[:, p]
                    nc.vector.copy_predicated(
                        out=from_sg,
                        mask=mk_p,
                        data=emb_sb[:, 2 * (p + 1) : 2 * (p + 1) + 1]
                        .unsqueeze(2)
                        .broadcast_to((P, NSQ, gcnt)),
                    )
                    nc.vector.copy_predicated(
                        out=to_sg,
                        mask=mk_p,
                        data=emb_sb[:, 2 * (p + 1) + 1 : 2 * (p + 1) + 2]
                        .unsqueeze(2)
                        .broadcast_to((P, NSQ, gcnt)),
                    )
                return from_t, to_t

            last_pool = None
            memsets = []
            for mi, (g0, gcnt) in enumerate(MEGAS):
                piece = MEGA_PIECE[mi]
                pg0 = g0 - OBS_PIECES[piece][0]
                obs4 = obs_tiles[piece][:, pg0 : pg0 + gcnt]
                from_t, to_t = emit_chain(gcnt, obs4)
                fr3 = from_t[:].rearrange("p (g f) -> p g f", g=gcnt)
                to3 = to_t[:].rearrange("p (g t) -> p g t", g=gcnt)

                ltd = lpool.tile([P, gcnt * DCOL], F32)
                ltp = pspool.tile([P, gcnt * PCOL], F32)
                ld3 = ltd[:].rearrange("p (g c) -> p g c", g=gcnt)
                lp3 = ltp[:].rearrange("p (g c) -> p g c", g=gcnt)
                # guard memsets: first writer of a REUSED slot takes the
                # WAR wait (old out-DMA completion sem) alone; only needed
                # once slots recycle (mi >= bufs)
                if mi >= 2:
                    nc.vector.memset(ltd[0:1, 0:1], 0.0)
                    nc.gpsimd.memset(
                        ltp[0:1, NPF * NSQ : NPF * NSQ + 1], 0.0
                    )

                nc.vector.tensor_tensor(
                    out=ld3.rearrange("p g (f t) -> p g f t", f=FSPLIT),
                    in0=fr3[:, :, 0:FSPLIT]
                    .unsqueeze(3)
                    .broadcast_to((P, gcnt, FSPLIT, NSQ)),
                    in1=to3.unsqueeze(2).broadcast_to((P, gcnt, FSPLIT, NSQ)),
                    op=mybir.AluOpType.add,
                )
                nc.gpsimd.tensor_tensor(
                    out=lp3[:, :, 0 : NPF * NSQ].rearrange(
                        "p g (f t) -> p g f t", f=NPF
                    ),
                    in0=fr3[:, :, FSPLIT:]
                    .unsqueeze(3)
                    .broadcast_to((P, gcnt, NPF, NSQ)),
                    in1=to3.unsqueeze(2).broadcast_to((P, gcnt, NPF, NSQ)),
                    op=mybir.AluOpType.add,
                )
                # diagonal zeros: f < FSPLIT live in ltd, the rest in ltp
                memsets.append(nc.vector.memset(ld3[:, :, 0:DCOL:65], 0.0))
                diag0 = FSPLIT * 65 - DCOL
                nc.gpsimd.memset(lp3[:, :, diag0 : NPF * NSQ : 65], 0.0)

                pr = ppool.tile([P, gcnt], F32)
                nc.vector.tensor_reduce(
                    out=pr[:],
                    in_=fr3,
                    axis=mybir.AxisListType.X,
                    op=mybir.AluOpType.max,
                )
                last_pool = nc.gpsimd.tensor_copy(
                    out=lp3[:, :, NPF * NSQ : PCOL],
                    in_=pr[:].unsqueeze(2).broadcast_to((P, gcnt, 4)),
                )

                # DVE-region out via scalar-engine HWDGE, POOL-region out
                # via gpsimd SWDGE — each ring stays within its 8 sem lanes
                all_dmas.append(
                    nc.scalar.dma_start(
                        out=log_v[:, g0 : g0 + gcnt, 0:DCOL], in_=ld3
                    )
                )
                all_dmas.append(
                    nc.gpsimd.dma_start(
                        out=log_v[:, g0 : g0 + gcnt, DCOL:NLOG], in_=lp3
                    )
                )

            # Tail: absorb each DMA-completion sem into its own SP nop
            # (the kernel-tail drain may carry only one wait itself)
            for d in all_dmas:
                n = nc.sync.nop(nofuse=True)
                bass_rust.add_dep_helper(
                    n.ins, d.ins, sync=True, reason="split tail drain waits"
                )
            for last in (memsets[-1], last_pool):
                n = nc.sync.nop(nofuse=True)
                bass_rust.add_dep_helper(
                    n.ins, last.ins, sync=True, reason="split tail drain waits"
                )
    return nc


_NC_CACHE = None


def _get_nc() -> bass.Bass:
    global _NC_CACHE
    if _NC_CACHE is None:
        _NC_CACHE = build_nc()
    return _NC_CACHE


def run(obs: np.ndarray, emb: np.ndarray, **spmd_kwargs):
    """Run the kernel; returns (logits, values, BassKernelResults)."""
    obs = np.ascontiguousarray(obs, dtype=np.float32).reshape(B_FULL, OBS_ROW)
    embrep = np.ascontiguousarray(
        np.broadcast_to(np.asarray(emb, dtype=np.float32).reshape(1, 26), (P, 26))
    )
    nc = _get_nc()
    in_maps = [
        {"obs": obs[i * B_CORE : (i + 1) * B_CORE], "embrep": embrep}
        for i in range(N_CORES)
    ]
    res = run_bass_kernel_spmd(nc, in_maps, list(range(N_CORES)), **spmd_kwargs)
    logits = np.concatenate([res.results[i]["logits"] for i in range(N_CORES)], axis=0)
    values = np.concatenate([res.results[i]["values"] for i in range(N_CORES)], axis=0)
    return logits, values, res


def kernel(obs: np.ndarray, emb: np.ndarray):
    logits, values, _ = run(obs, emb)
    return logits, values
